# revision 43
# baseline (speedup 1.0000x reference)
"""BlockwiseQuantLinear on 8 trn2 NeuronCores.

y = act_quant_dequant(x) @ (fp8_weight * block_scales).T
  x: [8192, 2048] f32, weight: [2048, 2048] fp8_e4m3fn (OCP), w_scale: [16, 16] f32
  out: [8192, 2048] f32

Strategy (data-parallel over tokens; hardcoded shapes):
  - Host: dequantize the static weight to fp16 (exact wrt reference up to fp16
    rounding), pre-transpose K-major as [wc, ki, j, n]; 4 chunk DMAs land
    [128 ki, 4 kb, 2048 n] in SBUF with 16KB-contiguous rows. Shard x rows 8
    ways. Output stored fp16 on device, widened to f32 on host.
  - Device (per core, M_sh=1024): weights fully SBUF-resident (64KB/partition),
    loaded on the scalar HWDGE ring in kb order. Per 128-row x tile: 1MB load
    (sync ring); blockwise act quant per (1,128) k-block: absmax reduce, EPS
    clip, reciprocal and fp8e4 quantize-mult on DVE (scale 224/amax: the TRN
    fp8e4 grid at half scale bit-matches the reference's OCP e4m3fn
    quantization), the x224 / /224 scale ops on ACT (Copy activation with
    scale), and the fp16 dequant-mult on GpSimd — splitting the chain keeps
    the DVE queue, the producer critical path, at ~7us per tile.
  - xbar transposes (both 1024-wide halves) on the sync ring ONLY. Two
    lessons learned on hardware: (1) two concurrent xbar transposes (one per
    HWDGE ring) interfere in the shared S2M xbar and corrupt the last source
    row of 16-row xbar tiles; (2) each transpose is a global DMA BARRIER —
    it waits for every previously-committed DMA's transfer and blocks later
    ones, so big transfers must not be committed between a transpose and its
    data-ready time (hence the load-after-transpose scheduling edges, and
    weights on the other ring).
  - Matmul stream: K-contiguous per m-tile — for kb in 16: for c in 4:
    psum[c] += xT[kb].T @ w[kb, c]. Stationary reused across the 4 n-chunk
    matmuls, 8 psum banks double-buffer across m-tiles, and the PE sees one
    long back-to-back stream so the HAM p-state holds 2.4GHz (the original
    per-matmul bank cycling oscillated at 1.2GHz). Warm-up matmuls on the
    identity cover the p-state ramp during the pipeline fill.
  - Emission is software-pipelined two m-tiles ahead (back(mi) after
    front(mi+2)) with widened tile rings, so every tile's transposes land a
    full matmul-stream (~14us) before the PE reads them — the transpose
    completion semaphore is a FixedSemInc(+16) lane that can be satisfied
    early by later DMAs' increments when consumed just-in-time.
  - Explicit scheduler edges (sync=True; sync=False hints are ignored): per-
    tile DVE order (reduce(mi) after quantize(mi-1), else the scheduler
    front-runs reduces and gridlocks the fill), and load(mi) after
    transpose(mi-2) (else all 8 loads commit ahead of the first transpose,
    whose barrier then waits out the WAR-stalled last load).
  - Last tile stores per n-chunk right after each bank evict to shorten the
    tail; other tiles store one [128, 2048] row block on the scalar ring.
  - Gather: concatenate the 8 row shards, astype(f32).
"""

import numpy as np
import ml_dtypes

import concourse.bass as bass
import concourse.mybir as mybir
import concourse.tile as tile
from concourse import bacc
from concourse.bass_utils import run_bass_kernel_spmd
from concourse.masks import make_identity

P = 128
M, K, N = 8192, 2048, 2048
NCORES = 8
M_SH = M // NCORES            # 1024 rows per core
MT = M_SH // P                # 8 m-tiles per core
KB = K // P                   # 16 k blocks
H = 2                         # halves per m-tile (quant/transpose granularity)
KBH = KB // H                 # 8 k blocks per half
KH_W = KBH * P                # 1024
NCH = 4                       # n chunks of 512
NC_W = N // NCH               # 512
WCH = 4                       # weight dma chunks of 4 kb each
EPS = 1e-12
N_WARM = 160                  # warm-up matmuls ([128,128] each)
LAG = 2                       # back(mi) emitted after front(mi+LAG)

_cache = {}


def _build():
    nc = bacc.Bacc(None, target_bir_lowering=False, num_swdge_queues=1)

    x_in = nc.dram_tensor("x_sh", [M_SH, K], mybir.dt.float32, kind="ExternalInput")
    # [wc, ki, j, n]: chunk wc holds k-blocks kb=4*wc+j, 16KB contiguous rows
    w_in = nc.dram_tensor(
        "wT", [WCH, P, KB // WCH, N], mybir.dt.float16, kind="ExternalInput"
    )
    y_out = nc.dram_tensor("y_sh", [M_SH, N], mybir.dt.float16, kind="ExternalOutput")

    with tile.TileContext(nc) as tc:
        with (
            tc.tile_pool(name="wpool", bufs=1) as wpool,
            tc.tile_pool(name="xpool", bufs=4) as xpool,
            tc.tile_pool(name="qpool", bufs=6) as qpool,
            tc.tile_pool(name="tpool", bufs=4) as tpool,
            tc.tile_pool(name="spool", bufs=4) as spool,
            tc.tile_pool(name="ypool", bufs=3) as ypool,
            tc.tile_pool(name="ps", bufs=2, space="PSUM") as ps,
        ):
            ident = spool.tile([P, P], mybir.dt.float16, name="ident", bufs=1)
            make_identity(nc, ident[:])

            wts = wpool.tile([P, KB, N], mybir.dt.float16, name="wts")

            # all weight chunks on the scalar ring, in kb order (kb-pair p
            # is first needed at mm0_start + 1.7us*p); 1MB granularity so
            # mi0's accumulation never waits long for the tail of a chunk.
            # Keeping them off the sync ring keeps them out of the
            # transposes' ring-drain path.
            for c8 in range(KB // 2):
                nc.scalar.dma_start(
                    wts[:, bass.ts(c8, 2), :],
                    w_in[c8 // 2][:, bass.ts(c8 % 2, 2), :],
                )

            # warm-up matmuls: keep the PE HAM activity window full while the
            # first x tile loads/quantizes, so real matmuls start at 2.4GHz.
            # Drawn from the psc0 tag so the 4 double-buffered chunk tags use
            # exactly the 8 PSUM banks (bufs are per-tag).
            warm_ps = ps.tile([P, NC_W], mybir.dt.float32, name="psc0", bufs=2)
            for _ in range(N_WARM):
                nc.tensor.matmul(
                    warm_ps[:, :P], ident[:], ident[:], start=True, stop=True
                )

            last_tt8 = [None]

            def quant(xg, t8, xdq, h, dve_dq=False):
                """Blockwise act-quant chain for half h: absmax per (1,128)
                block -> exact-match fp8 quantize (224 trick) -> fp16 dequant
                (on GpSimd: its rounding only affects the fp16 approximation,
                not the reference-matching fp8 grid)."""
                x3 = xg[:, bass.ts(h, KH_W)].rearrange(
                    "p (kb ki) -> p kb ki", kb=KBH
                )
                # one consolidated scale tile per half (slices: 0=amax,
                # 1=amaxp, 2=rec, 3=inv2, 4=s2) — fewer tile buffers means
                # fewer semaphores, which shortens both runtime waits and the
                # ~290-instruction semaphore teardown at the end of the NEFF
                sc = spool.tile([P, 5, KBH], mybir.dt.float32,
                                name=f"sc{h}", bufs=4)
                rd = nc.vector.tensor_reduce(
                    sc[:, 0, :], x3, axis=mybir.AxisListType.X,
                    op=mybir.AluOpType.max, apply_absolute_value=True,
                )
                if last_tt8[0] is not None:
                    # pin per-tile DVE order (real dep; sync=False hints are
                    # ignored by the scheduler)
                    tile.add_dep_helper(
                        rd.ins, last_tt8[0].ins, sync=True,
                        reason="per-tile DVE order",
                    )
                nc.vector.tensor_scalar_max(sc[:, 1, :], sc[:, 0, :], EPS)
                nc.vector.reciprocal(sc[:, 2, :], sc[:, 1, :])
                # the x224 / /224 scale ops run on ACT (Copy activation with
                # scale) to keep the DVE queue short
                nc.scalar.activation(
                    sc[:, 3, :], sc[:, 2, :],
                    mybir.ActivationFunctionType.Copy, scale=224.0
                )
                nc.scalar.activation(
                    sc[:, 4, :], sc[:, 1, :],
                    mybir.ActivationFunctionType.Copy, scale=1.0 / 224.0,
                )

                t83 = t8[:, bass.ts(h, KH_W)].rearrange("p (kb ki) -> p kb ki", kb=KBH)
                last_tt8[0] = nc.vector.tensor_tensor(
                    t83, x3, sc[:, 3, :, None].to_broadcast([P, KBH, P]),
                    mybir.AluOpType.mult,
                )
                xdq3 = xdq[:, bass.ts(h, KH_W)].rearrange(
                    "p (kb ki) -> p kb ki", kb=KBH
                )
                # tile 0's dequant runs on the (then-idle) DVE: it finishes
                # ~1us earlier than GpSimd and directly gates the first PE
                # transpose and therefore mm0
                dq_eng = nc.vector if dve_dq else nc.gpsimd
                dq_eng.tensor_tensor(
                    xdq3, t83, sc[:, 4, :, None].to_broadcast([P, KBH, P]),
                    mybir.AluOpType.mult,
                )

            xgs = {}
            xTs = {}
            xt_instrs = {}

            def load(mi):
                xg = xpool.tile([P, K], mybir.dt.float32, name="xg", bufs=4)
                ld = nc.sync.dma_start(xg[:], x_in[bass.ts(mi, P), :])
                if (mi - 2) in xt_instrs:
                    # keep the scheduler from committing every load ahead of
                    # the first transpose on the shared sync ring — the
                    # transpose's DMA barrier would then wait out the last
                    # load's WAR-stalled transfer
                    tile.add_dep_helper(
                        ld.ins, xt_instrs[mi - 2].ins, sync=True,
                        reason="sync-ring order: load after transpose",
                    )
                xgs[mi] = xg

            def front(mi):
                """Quant + transpose for m-tile mi."""
                xg = xgs.pop(mi)
                t8 = qpool.tile([P, K], mybir.dt.float8e4, name="t8", bufs=6)
                xdq = qpool.tile([P, K], mybir.dt.float16, name="xdq", bufs=6)
                xT = tpool.tile([P, KB, P], mybir.dt.float16, name="xT", bufs=4)
                for h in range(H):
                    quant(xg, t8, xdq, h, dve_dq=(mi == 0))
                    if mi == 0:
                        # PE-mode transposes for the first tile: a DMA
                        # transpose is a global DMA barrier, so it would wait
                        # out the whole 8MB weight preload before the PE can
                        # start. The PE path has no such barrier and runs in
                        # the fill window where the PE is idle anyway. One
                        # psum bank per half (psc1/psc2 tags), single ACT
                        # copy out.
                        tp = ps.tile([P, KH_W], mybir.dt.float16,
                                     name=f"psc{1 + h}", bufs=2)
                        for j in range(KBH):
                            xt_instrs[mi] = nc.tensor.transpose(
                                tp[:, bass.ts(j, P)],
                                xdq[:, bass.ts(h, KH_W)][:, bass.ts(j, P)],
                                ident[:],
                            )
                        nc.scalar.copy(
                            xT[:, bass.ts(h, KBH), :].rearrange(
                                "p a b -> p (a b)"
                            ),
                            tp[:],
                        )
                    else:
                        # ALL xbar transposes on the sync ring: two
                        # concurrent xbar transposes (one per HWDGE ring)
                        # interfere in the shared S2M xbar and corrupt data.
                        # One ring = FIFO = never concurrent.
                        xt_instrs[mi] = nc.sync.dma_start_transpose(
                            xT[:, bass.ts(h, KBH), :], xdq[:, bass.ts(h, KH_W)]
                        )
                xTs[mi] = xT

            def back(mi):
                """K-contiguous matmul stream + evict + store for m-tile mi."""
                xT = xTs.pop(mi)
                pss = [
                    ps.tile([P, NC_W], mybir.dt.float32, name=f"psc{c}", bufs=2)
                    for c in range(NCH)
                ]
                for kb in range(KB):
                    for c in range(NCH):
                        nc.tensor.matmul(
                            pss[c][:], xT[:, kb, :], wts[:, kb, bass.ts(c, NC_W)],
                            start=(kb == 0), stop=(kb == KB - 1),
                        )
                yt = ypool.tile([P, N], mybir.dt.float16, name="yt", bufs=3)
                if mi == MT - 1:
                    # shorten the tail: store each chunk right after its
                    # evict, on the (by now idle) sync ring so the store
                    # issues don't queue behind the evict copies on ACT
                    for c in range(NCH):
                        nc.scalar.copy(yt[:, bass.ts(c, NC_W)], pss[c][:])
                        nc.sync.dma_start(
                            y_out[bass.ts(mi, P), bass.ts(c, NC_W)],
                            yt[:, bass.ts(c, NC_W)],
                        )
                else:
                    # evicts explicitly on ACT ('any' would put some on the
                    # DVE, whose queue is the producer critical path)
                    for c in range(NCH):
                        nc.scalar.copy(yt[:, bass.ts(c, NC_W)], pss[c][:])
                    nc.scalar.dma_start(y_out[bass.ts(mi, P), :], yt[:])

            # software-pipelined emission, LAG tiles of slack between a
            # tile's transposes and its matmul stream
            load(0)
            load(1)
            for step in range(MT + LAG):
                if step < MT:
                    front(step)
                if step + 2 < MT:
                    load(step + 2)
                if step >= LAG:
                    back(step - LAG)

    nc.compile()
    return nc


def _prep_weight(weight: np.ndarray, w_scale: np.ndarray) -> np.ndarray:
    w_f32 = weight.astype(np.float32)                     # exact
    ws_full = np.repeat(np.repeat(w_scale.astype(np.float32), P, axis=0), P, axis=1)
    w_deq = (w_f32 * ws_full).astype(np.float16)          # [N, K]
    # w_deq.T[k, n]: k = (wc*4 + j)*128 + ki -> [wc, ki, j, n]
    wt = np.ascontiguousarray(
        w_deq.T.reshape(WCH, KB // WCH, P, N).transpose(0, 2, 1, 3)
    )
    return wt


def kernel(x: np.ndarray, weight: np.ndarray, w_scale: np.ndarray, _trace: bool = False):
    if "nc" not in _cache:
        _cache["nc"] = _build()
    nc = _cache["nc"]

    weight = np.asarray(weight)
    w_scale = np.asarray(w_scale, dtype=np.float32)
    wt = _prep_weight(weight, w_scale)
    x = np.ascontiguousarray(np.asarray(x), dtype=np.float32)

    in_maps = [
        {"x_sh": x[c * M_SH:(c + 1) * M_SH], "wT": wt}
        for c in range(NCORES)
    ]
    res = run_bass_kernel_spmd(
        nc, in_maps, core_ids=list(range(NCORES)),
        trace=_trace, trace_cores=list(range(NCORES)) if _trace else None,
    )
    y = np.concatenate(
        [res.results[c]["y_sh"] for c in range(NCORES)], axis=0
    ).astype(np.float32)
    if _trace:
        kernel.last_results = res
    return y
